# revision 39
# baseline (speedup 1.0000x reference)
"""MultiHeadAttention Trainium2 Bass kernel (v2: engine-rebalanced).

Problem: B=4, S=2048, C=512, H=8, D=64 MHA with learned relative-position
bias table gathered by bias_idxs == ones(49,49).  That gather makes the
bias a per-head constant, which is invariant under softmax over the key
axis, so the bias path is mathematically a no-op and is dropped.

Sharding (8 cores): core c handles batch b = c//2 and head-group
g = c%2 (4 heads = 256 channels).  Wq/Wk/Wv are sharded on their output
dim, Wo on its input dim; the two head-group partial outputs per batch
are summed on the host (the post-projection all-reduce).

v2 changes vs v1 (evidence: cost-model timeline sim — span was
dependency-bound, not capacity-bound; ACT(exp) busy 134us is the floor):
  - bv folded into bo host-side (Wo @ bv is a constant).
  - Wq/bq pre-scaled by 1/128 host-side; exp uses ACT scale=16.
  - psum evacuations rebalanced onto the idle Pool/GPSIMD engine;
    reciprocal -> reciprocal_approx_fast; Z rows stacked into SBUF
    via tiny DMAs (no PE zstack matmuls, no eye4).
  - xT DMA split per token-chunk; first score group starts after only
    kT/qT slice-0 chunk-0 projections.
  - qc-0 projection weaving is strictly demand-driven (one group per
    score slot), qT(qc+1) is produced inside qc's loop, and the per-qc
    normalize/out-proj tail is deferred per-pair into later score slots
    so neither PE nor ACT head-of-line blocks.
"""

import numpy as np
import ml_dtypes

P = 128
S = 2048          # sequence
CIN = 512         # model dim
CG = 256          # channels per head-group (4 heads x 64)
D = 64            # head dim
NH = 4            # heads per group
QC = 512          # query chunk (psum bank)
NQC = S // QC     # 4
NKC = S // P      # 16 key chunks of 128

_CACHE = {}

# tuning knobs (also overridable for A/B benching)
EXP_SPLIT = 2      # every EXP_SPLIT-th exp runs on DVE (0 = all on ACT)
PSUM_MODE = "A"    # "A": sp bufs=3 + pv 2, tail shares sp ring
                   # "B": sp bufs=2 + pv 3 + dedicated tail bank


def _register_exp_op():
    """Register a custom DVE op computing exp(16*x) = (1 + x + x^2/2)^16
    for |x| << 1 (scores arrive pre-scaled by 1/128, so x = s_true/128 and
    |x| < ~0.03; Taylor-2 + 4 squarings is accurate to ~1e-5 rel).
    Exactly 8 pipeline stages: mul,add,mul,add + 4x square.  Lets the DVE
    absorb ~1/3 of the softmax exp stream that otherwise makes the ACT
    engine the kernel's critical path."""
    from concourse import dve_ops
    from concourse.dve_spec import Spec, Src0, C0, C1, C2, sq, lower
    from concourse.dve_spec import _has_src1 as has_src1
    from concourse.dve_uop import DveOpSpec

    name = "EXP_POLY16_ANT"
    if name in dve_ops._SUB_OPCODE_FOR_NAME:
        return next(op for op in dve_ops.OPS if op.name == name)

    def ref(in0, in1, s0, s1, imm2):
        x = in0.astype(np.float32)
        return ((x * imm2 + s1) * x + s0) ** 16

    spec = Spec(
        body=sq(sq(sq(sq((Src0 * C2 + C1) * Src0 + C0)))),
        reference=ref,
    )
    row = dve_ops._CUSTOM_DVE_ROW_BASE + len(dve_ops.OPS)
    assert row < 0x20
    shas = {}
    for ver in ("v3", "v4"):
        try:
            tmp = DveOpSpec(name=name, opcode=row,
                            uops=lower(spec, ver=ver),
                            rd1_en=has_src1(spec))
            shas[ver] = tmp.sha(ver)
        except Exception:
            pass
    op = dve_ops.DveOp(name, spec, subdim=False, uops_sha=shas)
    dve_ops.OPS.append(op)
    dve_ops.CUSTOM_DVE_SPECS[name] = spec
    dve_ops._SUB_OPCODE_FOR_NAME[name] = row
    return op


def _build_nc(loop_n=1):
    import contextlib
    import concourse.tile as tile
    from concourse import bacc, mybir

    bf16 = mybir.dt.bfloat16
    f32 = mybir.dt.float32
    f16 = mybir.dt.float16

    exp_op = _register_exp_op()
    nc = bacc.Bacc("TRN2", target_bir_lowering=False, debug=False, num_devices=8)

    xT = nc.dram_tensor("xT", [CIN, S], bf16, kind="ExternalInput")
    wqT = nc.dram_tensor("wqT", [CIN, CG], bf16, kind="ExternalInput")
    wkT = nc.dram_tensor("wkT", [CIN, CG], bf16, kind="ExternalInput")
    wvT = nc.dram_tensor("wvT", [CIN, CG], bf16, kind="ExternalInput")
    woT = nc.dram_tensor("woT", [CG, CIN], bf16, kind="ExternalInput")
    bq = nc.dram_tensor("bq", [CG], f32, kind="ExternalInput")
    bk = nc.dram_tensor("bk", [CG], f32, kind="ExternalInput")
    bo = nc.dram_tensor("bo", [CIN], f32, kind="ExternalInput")
    # 2-row broadcast selector (partition bases must be 32-aligned, so this
    # can't be built with per-row memsets on chip): sel2[:, i*P:(i+1)*P] has
    # row i all-ones -> matmul broadcasts rz row i across 128 partitions.
    sel2 = nc.dram_tensor("sel2", [2, 2 * P], f16, kind="ExternalInput")
    outT = nc.dram_tensor("outT", [CIN, S], bf16, kind="ExternalOutput")

    with tile.TileContext(nc) as tc:
        # bench-only: repeat the whole body on-device to amplify exec time
        # above the PJRT dispatch noise floor
        loop_cm = tc.For_i(0, loop_n, 1) if loop_n > 1 else contextlib.nullcontext()
        with loop_cm, \
             tc.tile_pool(name="const", bufs=1) as const, \
             tc.tile_pool(name="big", bufs=1) as big, \
             tc.tile_pool(name="pt", bufs=4) as ptp, \
             tc.tile_pool(name="zs", bufs=4) as zsp, \
             tc.tile_pool(name="rzstage", bufs=4) as rzsp, \
             tc.tile_pool(name="spool", bufs=(3 if PSUM_MODE == "A" else 2),
                          space="PSUM") as sp, \
             tc.tile_pool(name="pvpool", bufs=(2 if PSUM_MODE == "A" else 3),
                          space="PSUM") as pvp, \
             tc.tile_pool(name="tailp", bufs=1, space="PSUM") as tp:

            def sp_tile():
                return sp.tile([P, 2, QC], mybir.dt.float32, tag="s", name="spt")

            # tail psum (bc / out-proj): mode A shares the score ring (3 sp
            # bufs give the score->exp->PV pipeline a kcg of lookahead);
            # mode B uses a dedicated bank
            def tp_tile():
                if PSUM_MODE == "A":
                    return sp.tile([P, 2, QC], mybir.dt.float32, tag="s",
                                   name="tpt")
                return tp.tile([P, 1, QC], mybir.dt.float32, tag="t",
                               name="tpt")

            # ---------- load inputs (weights first, x per token-chunk) ----------
            wk_sb = big.tile([P, CIN // P, CG], bf16, tag="wk")
            nc.sync.dma_start(wk_sb[:], wkT.rearrange("(o p) c -> p o c", p=P))
            wq_sb = big.tile([P, CIN // P, CG], bf16, tag="wq")
            nc.sync.dma_start(wq_sb[:], wqT.rearrange("(o p) c -> p o c", p=P))
            bq_sb = const.tile([P, CG // P], f32, tag="bq")
            nc.sync.dma_start(bq_sb[:], bq.rearrange("(s p) -> p s", p=P))
            bk_sb = const.tile([P, CG // P], f32, tag="bk")
            nc.sync.dma_start(bk_sb[:], bk.rearrange("(s p) -> p s", p=P))
            xT_sb = big.tile([P, CIN // P, S], bf16, tag="xT")
            xT_r = xT.rearrange("(o p) t -> p o t", p=P)
            for t in range(NQC):
                tsl = slice(t * QC, (t + 1) * QC)
                nc.sync.dma_start(xT_sb[:, :, tsl], xT_r[:, :, tsl])
            wv_sb = big.tile([P, CIN // P, CG], bf16, tag="wv")
            nc.sync.dma_start(wv_sb[:], wvT.rearrange("(o p) c -> p o c", p=P))
            wo_sb = big.tile([P, CG // P, CIN], bf16, tag="wo")
            nc.sync.dma_start(wo_sb[:], woT.rearrange("(o p) c -> p o c", p=P))
            bo_sb = const.tile([P, CIN // P], f32, tag="bo")
            nc.sync.dma_start(bo_sb[:], bo.rearrange("(s p) -> p s", p=P))
            sel_sb = const.tile([2, 2 * P], f16, tag="sel2")
            nc.sync.dma_start(sel_sb[:], sel2[:])

            # ---------- projections ----------
            qT_sb = big.tile([P, CG // P, S], bf16, tag="qT")
            kT_sb = big.tile([P, CG // P, S], bf16, tag="kT")
            # v token-major with a ones column per head (for Z)
            v_sb = big.tile([P, NKC, NH, D + 1], bf16, tag="v")
            nc.vector.memset(v_sb[:], 1.0)

            # channel-major qT/kT projection for one (cout-slice, token range)
            def proj_qk(dst, w, b, s, t, half=None):
                pj = sp_tile()
                tsl = slice(t * QC + (0 if half == 1 else 0),
                            t * QC + (QC // 2 if half == 0 else QC))
                if half == 1:
                    tsl = slice(t * QC + QC // 2, (t + 1) * QC)
                n = tsl.stop - tsl.start
                for ci in range(CIN // P):
                    nc.tensor.matmul(
                        pj[:, 0, :n],
                        w[:, ci, s * P:(s + 1) * P],
                        xT_sb[:, ci, tsl],
                        start=(ci == 0),
                        stop=(ci == CIN // P - 1),
                    )
                nc.scalar.activation(
                    dst[:, s, tsl], pj[:, 0, :n],
                    mybir.ActivationFunctionType.Identity,
                    bias=b[:, s:s + 1], scale=1.0,
                )

            # token-major v for one 128-token slice and one head-pair (so
            # pair 1's v work overlaps pair 1's attention); bv is folded
            # into bo host-side (probs sum to 1 after normalization, so
            # Wo @ (ctx/Z + bv) = Wo @ (ctx/Z) + const)
            def proj_v(t, vp):
                pj = sp_tile()
                csl = slice(vp * 2 * D, (vp + 1) * 2 * D)
                for ci in range(CIN // P):
                    nc.tensor.matmul(
                        pj[:, 0, csl],
                        xT_sb[:, ci, t * P:(t + 1) * P],
                        wv_sb[:, ci, csl],
                        start=(ci == 0),
                        stop=(ci == CIN // P - 1),
                    )
                # GPSIMD can't read PSUM; ScalarE is the cheapest psum evac
                # and has idle capacity exactly here (qc0 starves on exp)
                nc.scalar.copy(
                    v_sb[:, t, 2 * vp:2 * (vp + 1), :D],
                    pj[:, 0, csl].rearrange("p (h d) -> p h d", d=D),
                )

            # Minimal prefix for the very first score group (kT on the first
            # half-chunk only), then weave the rest demand-driven into the
            # qc-0 score slots.
            proj_qk(kT_sb, wk_sb, bk_sb, 0, 0, half=0)
            proj_qk(qT_sb, wq_sb, bq_sb, 0, 0)

            def mk(kind, s, t, half=None):
                if kind == "v":
                    return lambda: proj_v(t, s)
                if kind == "k":
                    return lambda: proj_qk(kT_sb, wk_sb, bk_sb, s, t, half)
                return lambda: proj_qk(qT_sb, wq_sb, bq_sb, s, t)

            # keys: ("v", vp, t) / ("k", s, tkey) / ("q", s, t); a kT item
            # with tkey covers key columns up to (tkey+1)*QC
            pend = []   # (key, emit_fn) in drain order
            pend.append((("v", 0, 0), mk("v", 0, 0)))
            pend.append((("v", 0, 1), mk("v", 0, 1)))
            pend.append((("k", 0, 0.5), mk("k", 0, 0, 1)))
            for t in range(1, NQC):
                for vt in (4 * t - 2, 4 * t - 1, 4 * t, 4 * t + 1):
                    if vt < NKC:
                        pend.append((("v", 0, vt), mk("v", 0, vt)))
                pend.append((("k", 0, t), mk("k", 0, t)))
            for vt in (NKC - 2, NKC - 1):
                pend.append((("v", 0, vt), mk("v", 0, vt)))
            # pair-1 items: its kT/qT slices and the v halves for heads 2,3
            pend.append((("k", 1, 0), mk("k", 1, 0)))
            pend.append((("q", 1, 0), mk("q", 1, 0)))
            for t in range(NQC):
                for vt in (4 * t, 4 * t + 1, 4 * t + 2, 4 * t + 3):
                    pend.append((("v", 1, vt), mk("v", 1, vt)))
                if t + 1 < NQC:
                    pend.append((("k", 1, t + 1), mk("k", 1, t + 1)))
            # dedup (keep first occurrence)
            seen = set()
            pend = [p for p in pend if not (p[0] in seen or seen.add(p[0]))]

            def proj_ensure(pair, kcg, phase):
                t_need = (2 * kcg + 1) / 4.0
                v_need = 2 * kcg + 1

                def needed(key):
                    if key[0] == "v":
                        return (phase == "pv" and key[1] == pair
                                and key[2] <= v_need)
                    if phase != "scores":
                        return False
                    if key[0] == "k":
                        return key[1] == pair and key[2] <= t_need
                    return key[1] == pair   # qT slice for this pair
                while pend and any(needed(k) for k, _ in pend):
                    k, f = pend.pop(0)
                    f()

            def proj_drain(n=1, allow_pair1=True):
                for _ in range(n):
                    if not pend:
                        return
                    if not allow_pair1 and pend[0][0][1] == 1:
                        return
                    pend.pop(0)[1]()

            # ---------- attention ----------
            ctx_raw = big.tile([P, CG // P, S], bf16, tag="ctxr")
            ctx_nrm = big.tile([P, CG // P, S], bf16, tag="ctxn")
            outT_sb = big.tile([P, CIN // P, S], bf16, tag="outT")

            # deferred work (closures), emitted one-per-score-slot so PE
            # never head-of-line blocks on tail dependencies
            tail_q = []
            exp_idx = [0]

            def emit_tail_some(n=1):
                for _ in range(n):
                    if tail_q:
                        tail_q.pop(0)()

            def emit_tail_all():
                while tail_q:
                    tail_q.pop(0)()

            def make_pair_tail(qc, pair, zstack):
                """normalize pair's two heads: recip(Z) -> broadcast -> mult"""
                qsl = slice(qc * QC, (qc + 1) * QC)
                items = []
                rz_f32 = rzsp.tile([2, QC], mybir.dt.float32, tag="rzf")
                rz_t = rzsp.tile([2, QC], mybir.dt.float16, tag="rz")

                def t_recip():
                    with nc.allow_low_precision(
                            reason="1/Z approx (~18 bits): Z ~ O(2048)"):
                        nc.vector.reciprocal_approx_fast(rz_f32[:], zstack[:])
                        nc.gpsimd.tensor_copy(rz_t[:], rz_f32[:])
                items.append(t_recip)

                for i in range(2):
                    def t_norm(i=i):
                        h = 2 * pair + i
                        hp, hs = D * (h % 2), h // 2
                        bc = tp_tile()
                        nc.tensor.matmul(
                            bc[:, 0, :],
                            sel_sb[:, i * P:(i + 1) * P],
                            rz_t[:],
                            start=True, stop=True,
                        )
                        sl = (slice(hp, hp + D), hs, qsl)
                        nc.vector.tensor_tensor(
                            ctx_nrm[sl], ctx_raw[sl], bc[hp:hp + D, 0, :],
                            mybir.AluOpType.mult,
                        )
                    items.append(t_norm)
                return items

            def make_out_tail(qc, last=False):
                qsl = slice(qc * QC, (qc + 1) * QC)
                items = []
                for oc in range(CIN // P):
                    def t_oproj(oc=oc):
                        # last qc: pv banks are free, rotate through them so
                        # the final (unoverlapped) tail pipelines
                        if last:
                            opap = pvp.tile([P, QC], mybir.dt.float32,
                                            tag="pv", name="op")[:, :]
                        else:
                            opap = tp_tile()[:, 0, :]
                        for s in range(CG // P):
                            nc.tensor.matmul(
                                opap,
                                wo_sb[:, s, oc * P:(oc + 1) * P],
                                ctx_nrm[:, s, qsl],
                                start=(s == 0),
                                stop=(s == CG // P - 1),
                            )
                        nc.scalar.activation(
                            outT_sb[:, oc, qsl], opap,
                            mybir.ActivationFunctionType.Identity,
                            bias=bo_sb[:, oc:oc + 1], scale=1.0,
                        )
                        nc.sync.dma_start(
                            outT.rearrange("(o p) t -> p o t", p=P)[:, oc, qsl],
                            outT_sb[:, oc, qsl],
                        )
                    items.append(t_oproj)
                return items

            for qc in range(NQC):
                qsl = slice(qc * QC, (qc + 1) * QC)
                for pair in range(2):
                    pvs = [pvp.tile([P, QC], mybir.dt.float32, tag="pv",
                                    name=f"pv{i}") for i in range(2)]
                    # Z rows for this pair, stacked on partitions 0..1 in
                    # SBUF (via tiny DMAs; engines need 32-aligned bases)
                    zstack = zsp.tile([2, QC], mybir.dt.float32, tag="z")
                    for kcg in range(NKC // 2):
                        if qc == 0:
                            proj_ensure(pair, kcg, "scores")
                        # two kc chunks of scores in 64-row tiling mode:
                        # heads 2p/2p+1 sit at partition bases 0/64 -> array
                        # tiles T0/T8 execute their matmuls concurrently
                        sts = []
                        for j in range(2):
                            kc = 2 * kcg + j
                            st = sp_tile()
                            for i in range(2):
                                h = 2 * pair + i
                                hp, hs = D * (h % 2), h // 2
                                nc.tensor.matmul(
                                    st[:, i, :],
                                    kT_sb[hp:hp + D, hs, kc * P:(kc + 1) * P],
                                    qT_sb[hp:hp + D, hs, qsl],
                                    start=True, stop=True,
                                    tile_position=(hp, 0),
                                )
                            sts.append(st)
                        pts = []
                        for j in range(2):
                            pt = ptp.tile([P, 2, QC], bf16, tag="pt",
                                          name=f"pt{j}")
                            # scores arrive pre-scaled by 1/128 (Wq host
                            # prescale); 16*s = s_true/8.  Roughly 1/3 of
                            # the exp stream runs on the DVE via the custom
                            # poly op so ACT stops being the critical path
                            # (skip DVE in qc0 pair0: DVE is evac-busy and
                            # ACT starved there).
                            nonlocal_idx = exp_idx[0]
                            exp_idx[0] += 1
                            use_dve = (EXP_SPLIT > 0
                                       and nonlocal_idx % EXP_SPLIT == 1
                                       and not (qc == 0 and pair == 0))
                            if use_dve:
                                nc.vector._custom_dve(
                                    exp_op, out=pt[:], in0=sts[j][:],
                                    s0=1.0, s1=1.0, imm2=0.5,
                                )
                            else:
                                nc.scalar.activation(
                                    pt[:], sts[j][:],
                                    mybir.ActivationFunctionType.Exp,
                                    bias=0.0, scale=16.0,
                                )
                            pts.append(pt)
                        if qc == 0:
                            proj_ensure(pair, kcg, "pv")
                        for j in range(2):
                            kc = 2 * kcg + j
                            for i in range(2):
                                h = 2 * pair + i
                                nc.tensor.matmul(
                                    pvs[i][:D + 1, :],
                                    v_sb[:, kc, h, :],
                                    pts[j][:, i, :],
                                    start=(kc == 0),
                                    stop=(kc == NKC - 1),
                                )
                        if qc == 0:
                            proj_drain(1, allow_pair1=(pair == 1 or kcg >= 6))
                        emit_tail_some(2)
                    for i in range(2):
                        h = 2 * pair + i
                        hp, hs = D * (h % 2), h // 2
                        # stash Z (Pool) and unnormalized ctxT (DVE) in
                        # parallel, freeing the pv bank
                        z_row = zsp.tile([1, QC], mybir.dt.float32, tag="zr")
                        nc.scalar.copy(z_row[:], pvs[i][D:D + 1, :])
                        nc.sync.dma_start(zstack[i:i + 1, :], z_row[:])
                        nc.vector.tensor_copy(
                            ctx_raw[hp:hp + D, hs, qsl], pvs[i][:D, :],
                        )
                    if pair == 0 and qc + 1 < NQC:
                        # produce qT for the next qc during this qc's pair 1
                        for s in range(CG // P):
                            tail_q.append(mk("q", s, qc + 1))
                    tail_q.extend(make_pair_tail(qc, pair, zstack))
                tail_q.extend(make_out_tail(qc, last=(qc == NQC - 1)))

            emit_tail_all()

    nc.compile()
    return nc


def _get_nc():
    if "nc" not in _CACHE:
        _CACHE["nc"] = _build_nc()
    return _CACHE["nc"]


SCALE_Q = 1.0 / 128.0   # host prescale on Wq/bq; exp uses ACT scale=16


def make_in_maps(query_states, Wq, bq, Wk, bk, Wv, bv, Wo, bo):
    """Host-side shard + layout prep. core c: batch c//2, head-group c%2."""
    bf = ml_dtypes.bfloat16
    x = np.asarray(query_states, np.float32)
    B = x.shape[0]
    in_maps = []
    xT_all = [np.ascontiguousarray(x[b].T).astype(bf) for b in range(B)]
    Wo32 = np.asarray(Wo, np.float32)
    bv32 = np.asarray(bv, np.float32)
    bo32 = np.asarray(bo, np.float32)
    w_sl = {}
    for g in range(2):
        c0, c1 = CG * g, CG * (g + 1)
        bo_eff = (bo32.copy() if g == 0 else np.zeros(CIN, np.float32))
        bo_eff += Wo32[:, c0:c1] @ bv32[c0:c1]
        w_sl[g] = dict(
            wqT=np.ascontiguousarray(
                np.asarray(Wq, np.float32)[c0:c1, :].T * SCALE_Q).astype(bf),
            wkT=np.ascontiguousarray(np.asarray(Wk, np.float32)[c0:c1, :].T).astype(bf),
            wvT=np.ascontiguousarray(np.asarray(Wv, np.float32)[c0:c1, :].T).astype(bf),
            woT=np.ascontiguousarray(Wo32[:, c0:c1].T).astype(bf),
            bq=np.ascontiguousarray(np.asarray(bq, np.float32)[c0:c1] * SCALE_Q),
            bk=np.ascontiguousarray(np.asarray(bk, np.float32)[c0:c1]),
            bo=bo_eff,
        )
    sel2 = np.zeros((2, 2 * P), np.float16)
    for i in range(2):
        sel2[i, i * P:(i + 1) * P] = 1.0
    for c in range(8):
        b, g = c // 2, c % 2
        m = {"xT": xT_all[b], "sel2": sel2}
        m.update(w_sl[g])
        in_maps.append(m)
    return in_maps


def gather_output(results):
    """Sum head-group partials per batch and transpose back to [B, S, C]."""
    B = 4
    out = np.empty((B, S, CIN), np.float32)
    for b in range(B):
        acc = (results[2 * b]["outT"].astype(np.float32)
               + results[2 * b + 1]["outT"].astype(np.float32))
        out[b] = acc.T
    return out


def kernel(query_states, Wq, bq, Wk, bk, Wv, bv, Wo, bo,
           attention_biases=None, bias_idxs=None, **_unused):
    # attention_biases/bias_idxs: bias_idxs is ones(49,49), so the gathered
    # bias is constant per head -> softmax-invariant -> no-op. Unused.
    from concourse.bass_utils import run_bass_kernel_spmd
    nc = _get_nc()
    in_maps = make_in_maps(query_states, Wq, bq, Wk, bk, Wv, bv, Wo, bo)
    res = run_bass_kernel_spmd(nc, in_maps, core_ids=list(range(8)))
    return gather_output(res.results)


# revision 54
# speedup vs baseline: 1.1626x; 1.1626x over previous
"""MultiHeadAttention Trainium2 Bass kernel (v2: engine-rebalanced).

Problem: B=4, S=2048, C=512, H=8, D=64 MHA with learned relative-position
bias table gathered by bias_idxs == ones(49,49).  That gather makes the
bias a per-head constant, which is invariant under softmax over the key
axis, so the bias path is mathematically a no-op and is dropped.

Sharding (8 cores): core c handles batch b = c//2 and head-group
g = c%2 (4 heads = 256 channels).  Wq/Wk/Wv are sharded on their output
dim, Wo on its input dim; the two head-group partial outputs per batch
are summed on the host (the post-projection all-reduce).

v2 changes vs v1 (evidence: cost-model timeline sim — span was
dependency-bound, not capacity-bound; ACT(exp) busy 134us is the floor):
  - bv folded into bo host-side (Wo @ bv is a constant).
  - Wq/bq pre-scaled by 1/128 host-side; exp uses ACT scale=16.
  - psum evacuations rebalanced onto the idle Pool/GPSIMD engine;
    reciprocal -> reciprocal_approx_fast; Z rows stacked into SBUF
    via tiny DMAs (no PE zstack matmuls, no eye4).
  - xT DMA split per token-chunk; first score group starts after only
    kT/qT slice-0 chunk-0 projections.
  - qc-0 projection weaving is strictly demand-driven (one group per
    score slot), qT(qc+1) is produced inside qc's loop, and the per-qc
    normalize/out-proj tail is deferred per-pair into later score slots
    so neither PE nor ACT head-of-line blocks.
"""

import numpy as np
import ml_dtypes

P = 128
S = 2048          # sequence
CIN = 512         # model dim
CG = 256          # channels per head-group (4 heads x 64)
D = 64            # head dim
NH = 4            # heads per group
QC = 512          # query chunk (psum bank)
NQC = S // QC     # 4
NKC = S // P      # 16 key chunks of 128

_CACHE = {}

# tuning knobs (also overridable for A/B benching)
EXP_SPLIT = 2      # every EXP_SPLIT-th exp runs on DVE (0 = all on ACT)
PSUM_MODE = "A"    # "A": sp bufs=3 + pv 2, tail shares sp ring
                   # "B": sp bufs=2 + pv 3 + dedicated tail bank


def _register_exp_op():
    """Register a custom DVE op computing exp(16*x) = (1 + x + x^2/2)^16
    for |x| << 1 (scores arrive pre-scaled by 1/128, so x = s_true/128 and
    |x| < ~0.03; Taylor-2 + 4 squarings is accurate to ~1e-5 rel).
    Exactly 8 pipeline stages: mul,add,mul,add + 4x square.  Lets the DVE
    absorb ~1/3 of the softmax exp stream that otherwise makes the ACT
    engine the kernel's critical path."""
    from concourse import dve_ops
    from concourse.dve_spec import Spec, Src0, C0, C1, C2, sq, lower
    from concourse.dve_spec import _has_src1 as has_src1
    from concourse.dve_uop import DveOpSpec

    name = "EXP_POLY16_ANT"
    if name in dve_ops._SUB_OPCODE_FOR_NAME:
        return next(op for op in dve_ops.OPS if op.name == name)

    def ref(in0, in1, s0, s1, imm2):
        x = in0.astype(np.float32)
        return ((x * imm2 + s1) * x + s0) ** 16

    spec = Spec(
        body=sq(sq(sq(sq((Src0 * C2 + C1) * Src0 + C0)))),
        reference=ref,
    )
    row = dve_ops._CUSTOM_DVE_ROW_BASE + len(dve_ops.OPS)
    assert row < 0x20
    shas = {}
    for ver in ("v3", "v4"):
        try:
            tmp = DveOpSpec(name=name, opcode=row,
                            uops=lower(spec, ver=ver),
                            rd1_en=has_src1(spec))
            shas[ver] = tmp.sha(ver)
        except Exception:
            pass
    op = dve_ops.DveOp(name, spec, subdim=False, uops_sha=shas)
    dve_ops.OPS.append(op)
    dve_ops.CUSTOM_DVE_SPECS[name] = spec
    dve_ops._SUB_OPCODE_FOR_NAME[name] = row
    return op


def _build_nc(loop_n=1):
    import contextlib
    import concourse.tile as tile
    from concourse import bacc, mybir

    bf16 = mybir.dt.bfloat16
    f32 = mybir.dt.float32
    f16 = mybir.dt.float16

    exp_op = _register_exp_op()
    nc = bacc.Bacc("TRN2", target_bir_lowering=False, debug=False, num_devices=8)

    xT = nc.dram_tensor("xT", [CIN, S], bf16, kind="ExternalInput")
    wqT = nc.dram_tensor("wqT", [CIN, CG], bf16, kind="ExternalInput")
    wkT = nc.dram_tensor("wkT", [CIN, CG], bf16, kind="ExternalInput")
    wvT = nc.dram_tensor("wvT", [CIN, CG], bf16, kind="ExternalInput")
    woT = nc.dram_tensor("woT", [CG, CIN], bf16, kind="ExternalInput")
    bq = nc.dram_tensor("bq", [CG], f32, kind="ExternalInput")
    bk = nc.dram_tensor("bk", [CG], f32, kind="ExternalInput")
    bo = nc.dram_tensor("bo", [CIN], f32, kind="ExternalInput")
    # 2-row broadcast selector (partition bases must be 32-aligned, so this
    # can't be built with per-row memsets on chip): sel2[:, i*P:(i+1)*P] has
    # row i all-ones -> matmul broadcasts rz row i across 128 partitions.
    sel2 = nc.dram_tensor("sel2", [2, 2 * P], f16, kind="ExternalInput")
    outT = nc.dram_tensor("outT", [CIN, S], bf16, kind="ExternalOutput")

    with tile.TileContext(nc) as tc:
        # bench-only: repeat the whole body on-device to amplify exec time
        # above the PJRT dispatch noise floor
        loop_cm = tc.For_i(0, loop_n, 1) if loop_n > 1 else contextlib.nullcontext()
        with loop_cm, \
             tc.tile_pool(name="const", bufs=1) as const, \
             tc.tile_pool(name="big", bufs=1) as big, \
             tc.tile_pool(name="pt", bufs=6) as ptp, \
             tc.tile_pool(name="zs", bufs=4) as zsp, \
             tc.tile_pool(name="rzstage", bufs=4) as rzsp, \
             tc.tile_pool(name="spool", bufs=(3 if PSUM_MODE == "A" else 2),
                          space="PSUM") as sp, \
             tc.tile_pool(name="pvpool", bufs=(2 if PSUM_MODE == "A" else 3),
                          space="PSUM") as pvp, \
             tc.tile_pool(name="tailp", bufs=1, space="PSUM") as tp:

            def sp_tile():
                return sp.tile([P, 2, QC], mybir.dt.float32, tag="s", name="spt")

            # tail psum (bc / out-proj): mode A shares the score ring (3 sp
            # bufs give the score->exp->PV pipeline a kcg of lookahead);
            # mode B uses a dedicated bank
            def tp_tile():
                if PSUM_MODE == "A":
                    return sp.tile([P, 2, QC], mybir.dt.float32, tag="s",
                                   name="tpt")
                return tp.tile([P, 1, QC], mybir.dt.float32, tag="t",
                               name="tpt")

            # ---------- load inputs (weights first, x per token-chunk) ----------
            wk_sb = big.tile([P, CIN // P, CG], bf16, tag="wk")
            nc.sync.dma_start(wk_sb[:], wkT.rearrange("(o p) c -> p o c", p=P))
            wq_sb = big.tile([P, CIN // P, CG], bf16, tag="wq")
            nc.sync.dma_start(wq_sb[:], wqT.rearrange("(o p) c -> p o c", p=P))
            bq_sb = const.tile([P, CG // P], f32, tag="bq")
            nc.sync.dma_start(bq_sb[:], bq.rearrange("(s p) -> p s", p=P))
            bk_sb = const.tile([P, CG // P], f32, tag="bk")
            nc.sync.dma_start(bk_sb[:], bk.rearrange("(s p) -> p s", p=P))
            xT_sb = big.tile([P, CIN // P, S], bf16, tag="xT")
            xT_r = xT.rearrange("(o p) t -> p o t", p=P)
            for t in range(NQC):
                tsl = slice(t * QC, (t + 1) * QC)
                nc.sync.dma_start(xT_sb[:, :, tsl], xT_r[:, :, tsl])
            wv_sb = big.tile([P, CIN // P, CG], bf16, tag="wv")
            nc.sync.dma_start(wv_sb[:], wvT.rearrange("(o p) c -> p o c", p=P))
            wo_sb = big.tile([P, CG // P, CIN], bf16, tag="wo")
            nc.sync.dma_start(wo_sb[:], woT.rearrange("(o p) c -> p o c", p=P))
            bo_sb = const.tile([P, CIN // P], f32, tag="bo")
            nc.sync.dma_start(bo_sb[:], bo.rearrange("(s p) -> p s", p=P))
            sel_sb = const.tile([2, 2 * P], f16, tag="sel2")
            nc.sync.dma_start(sel_sb[:], sel2[:])

            # ---------- projections ----------
            qT_sb = big.tile([P, CG // P, S], bf16, tag="qT")
            kT_sb = big.tile([P, CG // P, S], bf16, tag="kT")
            # v token-major with a ones column per head (for Z)
            v_sb = big.tile([P, NKC, NH, D + 1], bf16, tag="v")
            nc.vector.memset(v_sb[:], 1.0)

            # channel-major qT/kT projection for one (cout-slice, token range)
            def proj_qk(dst, w, b, s, t, half=None):
                pj = sp_tile()
                tsl = slice(t * QC + (0 if half == 1 else 0),
                            t * QC + (QC // 2 if half == 0 else QC))
                if half == 1:
                    tsl = slice(t * QC + QC // 2, (t + 1) * QC)
                n = tsl.stop - tsl.start
                for ci in range(CIN // P):
                    nc.tensor.matmul(
                        pj[:, 0, :n],
                        w[:, ci, s * P:(s + 1) * P],
                        xT_sb[:, ci, tsl],
                        start=(ci == 0),
                        stop=(ci == CIN // P - 1),
                    )
                if EXP_SPLIT > 0:
                    # exps split onto DVE -> keep its FIFO thin: evac on ACT
                    nc.scalar.activation(
                        dst[:, s, tsl], pj[:, 0, :n],
                        mybir.ActivationFunctionType.Identity,
                        bias=b[:, s:s + 1], scale=1.0,
                    )
                else:
                    nc.vector.tensor_scalar_add(
                        dst[:, s, tsl], pj[:, 0, :n], b[:, s:s + 1],
                    )

            # token-major v for one 128-token slice and one head-pair (so
            # pair 1's v work overlaps pair 1's attention); bv is folded
            # into bo host-side (probs sum to 1 after normalization, so
            # Wo @ (ctx/Z + bv) = Wo @ (ctx/Z) + const)
            def proj_v(t, vp):
                pj = sp_tile()
                csl = slice(vp * 2 * D, (vp + 1) * 2 * D)
                for ci in range(CIN // P):
                    nc.tensor.matmul(
                        pj[:, 0, csl],
                        xT_sb[:, ci, t * P:(t + 1) * P],
                        wv_sb[:, ci, csl],
                        start=(ci == 0),
                        stop=(ci == CIN // P - 1),
                    )
                # GPSIMD can't read PSUM; ScalarE is the cheapest psum evac
                # and has idle capacity exactly here (qc0 starves on exp)
                nc.scalar.copy(
                    v_sb[:, t, 2 * vp:2 * (vp + 1), :D],
                    pj[:, 0, csl].rearrange("p (h d) -> p h d", d=D),
                )

            # Minimal prefix for the very first score group (kT on the first
            # half-chunk only), then weave the rest demand-driven into the
            # qc-0 score slots.
            proj_qk(kT_sb, wk_sb, bk_sb, 0, 0, half=0)
            proj_qk(qT_sb, wq_sb, bq_sb, 0, 0)

            def mk(kind, s, t, half=None):
                if kind == "v":
                    return lambda: proj_v(t, s)
                if kind == "k":
                    return lambda: proj_qk(kT_sb, wk_sb, bk_sb, s, t, half)
                return lambda: proj_qk(qT_sb, wq_sb, bq_sb, s, t)

            # keys: ("v", vp, t) / ("k", s, tkey) / ("q", s, t); a kT item
            # with tkey covers key columns up to (tkey+1)*QC
            pend = []   # (key, emit_fn) in drain order
            pend.append((("v", 0, 0), mk("v", 0, 0)))
            pend.append((("v", 0, 1), mk("v", 0, 1)))
            pend.append((("k", 0, 0.5), mk("k", 0, 0, 1)))
            for t in range(1, NQC):
                for vt in (4 * t - 2, 4 * t - 1, 4 * t, 4 * t + 1):
                    if vt < NKC:
                        pend.append((("v", 0, vt), mk("v", 0, vt)))
                pend.append((("k", 0, t), mk("k", 0, t)))
            for vt in (NKC - 2, NKC - 1):
                pend.append((("v", 0, vt), mk("v", 0, vt)))
            # pair-1 items: its kT/qT slices and the v halves for heads 2,3
            pend.append((("k", 1, 0), mk("k", 1, 0)))
            pend.append((("q", 1, 0), mk("q", 1, 0)))
            for t in range(NQC):
                for vt in (4 * t, 4 * t + 1, 4 * t + 2, 4 * t + 3):
                    pend.append((("v", 1, vt), mk("v", 1, vt)))
                if t + 1 < NQC:
                    pend.append((("k", 1, t + 1), mk("k", 1, t + 1)))
            # dedup (keep first occurrence)
            seen = set()
            pend = [p for p in pend if not (p[0] in seen or seen.add(p[0]))]

            def proj_ensure(pair, kcg, phase):
                t_need = (2 * kcg + 1) / 4.0
                v_need = 2 * kcg + 1

                def needed(key):
                    if key[0] == "v":
                        return (phase == "pv" and key[1] == pair
                                and key[2] <= v_need)
                    if phase != "scores":
                        return False
                    if key[0] == "k":
                        return key[1] == pair and key[2] <= t_need
                    return key[1] == pair   # qT slice for this pair
                while pend and any(needed(k) for k, _ in pend):
                    k, f = pend.pop(0)
                    f()

            def proj_drain(n=1, allow_pair1=True):
                for _ in range(n):
                    if not pend:
                        return
                    if not allow_pair1 and pend[0][0][1] == 1:
                        return
                    pend.pop(0)[1]()

            # ---------- attention ----------
            ctx_raw = big.tile([P, CG // P, S], bf16, tag="ctxr")
            ctx_nrm = big.tile([P, CG // P, S], bf16, tag="ctxn")
            outT_sb = big.tile([P, CIN // P, S], bf16, tag="outT")

            # deferred work (closures), emitted one-per-score-slot so PE
            # never head-of-line blocks on tail dependencies
            tail_q = []
            exp_idx = [0]

            def emit_tail_some(n=1):
                for _ in range(n):
                    if tail_q:
                        tail_q.pop(0)()

            def emit_tail_all():
                while tail_q:
                    tail_q.pop(0)()

            def make_pair_tail(qc, pair, zstack, zrows=None):
                """normalize pair's two heads: recip(Z) -> broadcast -> mult.
                With zrows (last pair): skip the z-stack DMAs (~1.5us sem
                latency each) and run per-row recips directly."""
                qsl = slice(qc * QC, (qc + 1) * QC)
                items = []
                nrow = 1 if zrows is not None else 2
                rz_f32 = [rzsp.tile([nrow, QC], mybir.dt.float32,
                                    tag=f"rzf{nrow}_{i}", name=f"rzf{i}")
                          for i in range(2 if zrows is not None else 1)]
                rz_t = [rzsp.tile([nrow, QC], mybir.dt.float16,
                                  tag=f"rz{nrow}_{i}", name=f"rz{i}")
                        for i in range(2 if zrows is not None else 1)]

                def t_recip():
                    with nc.allow_low_precision(
                            reason="1/Z approx (~18 bits): Z ~ O(2048)"):
                        if zrows is not None:
                            for i in range(2):
                                nc.vector.reciprocal_approx_fast(
                                    rz_f32[i][:], zrows[i][:])
                                nc.gpsimd.tensor_copy(rz_t[i][:], rz_f32[i][:])
                        else:
                            nc.vector.reciprocal_approx_fast(
                                rz_f32[0][:], zstack[:])
                            nc.gpsimd.tensor_copy(rz_t[0][:], rz_f32[0][:])
                items.append(t_recip)

                for i in range(2):
                    def t_norm(i=i):
                        h = 2 * pair + i
                        hp, hs = D * (h % 2), h // 2
                        bc = tp_tile()
                        if zrows is not None:
                            nc.tensor.matmul(
                                bc[:, 0, :],
                                sel_sb[0:1, 0:P],
                                rz_t[i][:],
                                start=True, stop=True,
                            )
                        else:
                            nc.tensor.matmul(
                                bc[:, 0, :],
                                sel_sb[:, i * P:(i + 1) * P],
                                rz_t[0][:],
                                start=True, stop=True,
                            )
                        sl = (slice(hp, hp + D), hs, qsl)
                        nc.vector.tensor_tensor(
                            ctx_nrm[sl], ctx_raw[sl], bc[hp:hp + D, 0, :],
                            mybir.AluOpType.mult,
                        )
                    items.append(t_norm)
                return items

            def make_out_tail(qc, last=False):
                qsl = slice(qc * QC, (qc + 1) * QC)
                items = []
                for oc in range(CIN // P):
                    def t_oproj(oc=oc):
                        # last qc: pv banks are free, rotate through them so
                        # the final (unoverlapped) tail pipelines
                        if last:
                            opap = pvp.tile([P, QC], mybir.dt.float32,
                                            tag="pv", name="op")[:, :]
                        else:
                            opap = tp_tile()[:, 0, :]
                        for s in range(CG // P):
                            nc.tensor.matmul(
                                opap,
                                wo_sb[:, s, oc * P:(oc + 1) * P],
                                ctx_nrm[:, s, qsl],
                                start=(s == 0),
                                stop=(s == CG // P - 1),
                            )
                        nc.scalar.activation(
                            outT_sb[:, oc, qsl], opap,
                            mybir.ActivationFunctionType.Identity,
                            bias=bo_sb[:, oc:oc + 1], scale=1.0,
                        )
                        nc.sync.dma_start(
                            outT.rearrange("(o p) t -> p o t", p=P)[:, oc, qsl],
                            outT_sb[:, oc, qsl],
                        )
                    items.append(t_oproj)
                return items

            for qc in range(NQC):
                qsl = slice(qc * QC, (qc + 1) * QC)
                for pair in range(2):
                    pvs = [pvp.tile([P, QC], mybir.dt.float32, tag="pv",
                                    name=f"pv{i}") for i in range(2)]
                    # Z rows for this pair, stacked on partitions 0..1 in
                    # SBUF (via tiny DMAs; engines need 32-aligned bases)
                    last_pair = (qc == NQC - 1 and pair == 1)
                    if not last_pair:
                        zstack = zsp.tile([2, QC], mybir.dt.float32, tag="z")
                    else:
                        zstack = None
                    # software pipeline: PV runs one kcg behind scores/exp so
                    # the PE never stalls on the exp round trip
                    prev_pv = None
                    for kcg in range(NKC // 2):
                        if qc == 0:
                            proj_ensure(pair, kcg, "scores")
                        # two kc chunks of scores in 64-row tiling mode:
                        # heads 2p/2p+1 sit at partition bases 0/64 -> array
                        # tiles T0/T8 execute their matmuls concurrently
                        sts = []
                        for j in range(2):
                            kc = 2 * kcg + j
                            st = sp_tile()
                            for i in range(2):
                                h = 2 * pair + i
                                hp, hs = D * (h % 2), h // 2
                                nc.tensor.matmul(
                                    st[:, i, :],
                                    kT_sb[hp:hp + D, hs, kc * P:(kc + 1) * P],
                                    qT_sb[hp:hp + D, hs, qsl],
                                    start=True, stop=True,
                                    tile_position=(hp, 0),
                                )
                            sts.append(st)
                        pts = []
                        for j in range(2):
                            pt = ptp.tile([P, 2, QC], bf16, tag="pt",
                                          name=f"pt{j}")
                            # scores arrive pre-scaled by 1/128 (Wq host
                            # prescale); 16*s = s_true/8.  Roughly 1/3 of
                            # the exp stream runs on the DVE via the custom
                            # poly op so ACT stops being the critical path
                            # (skip DVE in qc0 pair0: DVE is evac-busy and
                            # ACT starved there).
                            nonlocal_idx = exp_idx[0]
                            exp_idx[0] += 1
                            use_dve = (EXP_SPLIT > 0
                                       and nonlocal_idx % EXP_SPLIT == 1
                                       and not (qc == 0 and pair == 0))
                            if use_dve:
                                nc.vector._custom_dve(
                                    exp_op, out=pt[:], in0=sts[j][:],
                                    s0=1.0, s1=1.0, imm2=0.5,
                                )
                            else:
                                nc.scalar.activation(
                                    pt[:], sts[j][:],
                                    mybir.ActivationFunctionType.Exp,
                                    bias=0.0, scale=16.0,
                                )
                            pts.append(pt)
                        def emit_pv(pts, kcg):
                            for j in range(2):
                                kc = 2 * kcg + j
                                for i in range(2):
                                    h = 2 * pair + i
                                    nc.tensor.matmul(
                                        pvs[i][:D + 1, :],
                                        v_sb[:, kc, h, :],
                                        pts[j][:, i, :],
                                        start=(kc == 0),
                                        stop=(kc == NKC - 1),
                                    )
                        if prev_pv is not None:
                            if qc == 0:
                                proj_ensure(pair, prev_pv[1], "pv")
                            emit_pv(*prev_pv)
                        prev_pv = (pts, kcg)
                        if qc == 0:
                            proj_drain(1, allow_pair1=(pair == 1 or kcg >= 6))
                        emit_tail_some(2)
                    if qc == 0:
                        proj_ensure(pair, prev_pv[1], "pv")
                    emit_pv(*prev_pv)
                    zrows = []
                    for i in range(2):
                        h = 2 * pair + i
                        hp, hs = D * (h % 2), h // 2
                        # stash Z (ACT) and unnormalized ctxT (DVE) in
                        # parallel, freeing the pv bank
                        z_row = zsp.tile([1, QC], mybir.dt.float32, tag="zr")
                        nc.scalar.copy(z_row[:], pvs[i][D:D + 1, :])
                        zrows.append(z_row)
                        if not last_pair:
                            nc.sync.dma_start(zstack[i:i + 1, :], z_row[:])
                        nc.vector.tensor_copy(
                            ctx_raw[hp:hp + D, hs, qsl], pvs[i][:D, :],
                        )
                    if pair == 0 and qc + 1 < NQC:
                        # produce qT for the next qc during this qc's pair 1
                        for s in range(CG // P):
                            tail_q.append(mk("q", s, qc + 1))
                    tail_q.extend(make_pair_tail(
                        qc, pair, zstack, zrows=zrows if last_pair else None))
                tail_q.extend(make_out_tail(qc, last=(qc == NQC - 1)))

            emit_tail_all()

    nc.compile()
    return nc


def _get_nc():
    if "nc" not in _CACHE:
        _CACHE["nc"] = _build_nc()
    return _CACHE["nc"]


SCALE_Q = 1.0 / 128.0   # host prescale on Wq/bq; exp uses ACT scale=16


def make_in_maps(query_states, Wq, bq, Wk, bk, Wv, bv, Wo, bo):
    """Host-side shard + layout prep. core c: batch c//2, head-group c%2."""
    bf = ml_dtypes.bfloat16
    x = np.asarray(query_states, np.float32)
    B = x.shape[0]
    in_maps = []
    xT_all = [np.ascontiguousarray(x[b].T).astype(bf) for b in range(B)]
    Wo32 = np.asarray(Wo, np.float32)
    bv32 = np.asarray(bv, np.float32)
    bo32 = np.asarray(bo, np.float32)
    w_sl = {}
    for g in range(2):
        c0, c1 = CG * g, CG * (g + 1)
        bo_eff = (bo32.copy() if g == 0 else np.zeros(CIN, np.float32))
        bo_eff += Wo32[:, c0:c1] @ bv32[c0:c1]
        w_sl[g] = dict(
            wqT=np.ascontiguousarray(
                np.asarray(Wq, np.float32)[c0:c1, :].T * SCALE_Q).astype(bf),
            wkT=np.ascontiguousarray(np.asarray(Wk, np.float32)[c0:c1, :].T).astype(bf),
            wvT=np.ascontiguousarray(np.asarray(Wv, np.float32)[c0:c1, :].T).astype(bf),
            woT=np.ascontiguousarray(Wo32[:, c0:c1].T).astype(bf),
            bq=np.ascontiguousarray(np.asarray(bq, np.float32)[c0:c1] * SCALE_Q),
            bk=np.ascontiguousarray(np.asarray(bk, np.float32)[c0:c1]),
            bo=bo_eff,
        )
    sel2 = np.zeros((2, 2 * P), np.float16)
    for i in range(2):
        sel2[i, i * P:(i + 1) * P] = 1.0
    for c in range(8):
        b, g = c // 2, c % 2
        m = {"xT": xT_all[b], "sel2": sel2}
        m.update(w_sl[g])
        in_maps.append(m)
    return in_maps


def gather_output(results):
    """Sum head-group partials per batch and transpose back to [B, S, C]."""
    B = 4
    out = np.empty((B, S, CIN), np.float32)
    for b in range(B):
        acc = (results[2 * b]["outT"].astype(np.float32)
               + results[2 * b + 1]["outT"].astype(np.float32))
        out[b] = acc.T
    return out


def kernel(query_states, Wq, bq, Wk, bk, Wv, bv, Wo, bo,
           attention_biases=None, bias_idxs=None, **_unused):
    # attention_biases/bias_idxs: bias_idxs is ones(49,49), so the gathered
    # bias is constant per head -> softmax-invariant -> no-op. Unused.
    from concourse.bass_utils import run_bass_kernel_spmd
    nc = _get_nc()
    in_maps = make_in_maps(query_states, Wq, bq, Wk, bk, Wv, bv, Wo, bo)
    res = run_bass_kernel_spmd(nc, in_maps, core_ids=list(range(8)))
    return gather_output(res.results)


# revision 57
# speedup vs baseline: 1.2167x; 1.0465x over previous
"""MultiHeadAttention Trainium2 Bass kernel (v2: engine-rebalanced).

Problem: B=4, S=2048, C=512, H=8, D=64 MHA with learned relative-position
bias table gathered by bias_idxs == ones(49,49).  That gather makes the
bias a per-head constant, which is invariant under softmax over the key
axis, so the bias path is mathematically a no-op and is dropped.

Sharding (8 cores): core c handles batch b = c//2 and head-group
g = c%2 (4 heads = 256 channels).  Wq/Wk/Wv are sharded on their output
dim, Wo on its input dim; the two head-group partial outputs per batch
are summed on the host (the post-projection all-reduce).

Changes vs the original baseline (evidence: cost-model timeline sim +
interleaved same-process HW A/B; ACT(exp) busy ~134us was the span floor):
  - custom DVE op EXP_POLY16_ANT: exp(16x) = (1 + x + x^2/2)^16 (scores
    host-prescaled by 1/128 so |x| <= ~0.12); every other exp runs on the
    Vector engine, removing ACT as the critical path (EXP_SPLIT=2).
  - PV software-pipelined one kcg behind scores/exp: the PE no longer
    stalls ~0.8us per slot on the exp round trip (biggest single win).
  - bv folded into bo host-side (Wo @ bv is a constant);
    reciprocal -> reciprocal_approx_fast; Z rows stacked into SBUF via
    tiny DMAs (no PE zstack matmuls); last qc skips the stack entirely.
  - evacuations placed per engine balance (GPSIMD cannot read PSUM):
    qk/v/z on ScalarE, ctx copies + normalize mult + out-adds on DVE,
    rz f16 copies on Pool; ones-column memset covers only that slice.
  - xT DMA split per token-chunk; demand-driven qc-0 projection weaving
    (v projections split per head-pair across the two pair phases);
    qT(qc+1) produced inside qc; per-qc normalize/out-proj tail deferred
    into the next qc's score slots.  PSUM mode B (sp 2x2 banks + pv 3 +
    tail 1) won the HW A/B over mode A.
"""

import numpy as np
import ml_dtypes

P = 128
S = 2048          # sequence
CIN = 512         # model dim
CG = 256          # channels per head-group (4 heads x 64)
D = 64            # head dim
NH = 4            # heads per group
QC = 512          # query chunk (psum bank)
NQC = S // QC     # 4
NKC = S // P      # 16 key chunks of 128

_CACHE = {}

# tuning knobs (also overridable for A/B benching)
EXP_SPLIT = 2      # every EXP_SPLIT-th exp runs on DVE (0 = all on ACT)
PSUM_MODE = "B"    # "A": sp bufs=3 + pv 2, tail shares sp ring
                   # "B": sp bufs=2 + pv 3 + dedicated tail bank


def _register_exp_op():
    """Register a custom DVE op computing exp(16*x) = (1 + x + x^2/2)^16
    for |x| << 1 (scores arrive pre-scaled by 1/128, so x = s_true/128 and
    |x| < ~0.03; Taylor-2 + 4 squarings is accurate to ~1e-5 rel).
    Exactly 8 pipeline stages: mul,add,mul,add + 4x square.  Lets the DVE
    absorb ~1/3 of the softmax exp stream that otherwise makes the ACT
    engine the kernel's critical path."""
    from concourse import dve_ops
    from concourse.dve_spec import Spec, Src0, C0, C1, C2, sq, lower
    from concourse.dve_spec import _has_src1 as has_src1
    from concourse.dve_uop import DveOpSpec

    name = "EXP_POLY16_ANT"
    if name in dve_ops._SUB_OPCODE_FOR_NAME:
        return next(op for op in dve_ops.OPS if op.name == name)

    def ref(in0, in1, s0, s1, imm2):
        x = in0.astype(np.float32)
        return ((x * imm2 + s1) * x + s0) ** 16

    spec = Spec(
        body=sq(sq(sq(sq((Src0 * C2 + C1) * Src0 + C0)))),
        reference=ref,
    )
    row = dve_ops._CUSTOM_DVE_ROW_BASE + len(dve_ops.OPS)
    assert row < 0x20
    shas = {}
    for ver in ("v3", "v4"):
        try:
            tmp = DveOpSpec(name=name, opcode=row,
                            uops=lower(spec, ver=ver),
                            rd1_en=has_src1(spec))
            shas[ver] = tmp.sha(ver)
        except Exception:
            pass
    op = dve_ops.DveOp(name, spec, subdim=False, uops_sha=shas)
    dve_ops.OPS.append(op)
    dve_ops.CUSTOM_DVE_SPECS[name] = spec
    dve_ops._SUB_OPCODE_FOR_NAME[name] = row
    return op


def _build_nc(loop_n=1):
    import contextlib
    import concourse.tile as tile
    from concourse import bacc, mybir

    bf16 = mybir.dt.bfloat16
    f32 = mybir.dt.float32
    f16 = mybir.dt.float16

    exp_op = _register_exp_op()
    nc = bacc.Bacc("TRN2", target_bir_lowering=False, debug=False, num_devices=8)

    xT = nc.dram_tensor("xT", [CIN, S], bf16, kind="ExternalInput")
    wqT = nc.dram_tensor("wqT", [CIN, CG], bf16, kind="ExternalInput")
    wkT = nc.dram_tensor("wkT", [CIN, CG], bf16, kind="ExternalInput")
    wvT = nc.dram_tensor("wvT", [CIN, CG], bf16, kind="ExternalInput")
    woT = nc.dram_tensor("woT", [CG, CIN], bf16, kind="ExternalInput")
    bq = nc.dram_tensor("bq", [CG], f32, kind="ExternalInput")
    bk = nc.dram_tensor("bk", [CG], f32, kind="ExternalInput")
    bo = nc.dram_tensor("bo", [CIN], f32, kind="ExternalInput")
    # 2-row broadcast selector (partition bases must be 32-aligned, so this
    # can't be built with per-row memsets on chip): sel2[:, i*P:(i+1)*P] has
    # row i all-ones -> matmul broadcasts rz row i across 128 partitions.
    sel2 = nc.dram_tensor("sel2", [2, 2 * P], f16, kind="ExternalInput")
    outT = nc.dram_tensor("outT", [CIN, S], bf16, kind="ExternalOutput")

    with tile.TileContext(nc) as tc:
        # bench-only: repeat the whole body on-device to amplify exec time
        # above the PJRT dispatch noise floor
        loop_cm = tc.For_i(0, loop_n, 1) if loop_n > 1 else contextlib.nullcontext()
        with loop_cm, \
             tc.tile_pool(name="const", bufs=1) as const, \
             tc.tile_pool(name="big", bufs=1) as big, \
             tc.tile_pool(name="pt", bufs=6) as ptp, \
             tc.tile_pool(name="zs", bufs=4) as zsp, \
             tc.tile_pool(name="rzstage", bufs=4) as rzsp, \
             tc.tile_pool(name="spool", bufs=(3 if PSUM_MODE == "A" else 2),
                          space="PSUM") as sp, \
             tc.tile_pool(name="pvpool", bufs=(2 if PSUM_MODE == "A" else 3),
                          space="PSUM") as pvp, \
             tc.tile_pool(name="tailp", bufs=1, space="PSUM") as tp:

            def sp_tile():
                return sp.tile([P, 2, QC], mybir.dt.float32, tag="s", name="spt")

            # tail psum (bc / out-proj): mode A shares the score ring (3 sp
            # bufs give the score->exp->PV pipeline a kcg of lookahead);
            # mode B uses a dedicated bank
            def tp_tile():
                if PSUM_MODE == "A":
                    return sp.tile([P, 2, QC], mybir.dt.float32, tag="s",
                                   name="tpt")
                return tp.tile([P, 1, QC], mybir.dt.float32, tag="t",
                               name="tpt")

            # ---------- load inputs (weights first, x per token-chunk) ----------
            wk_sb = big.tile([P, CIN // P, CG], bf16, tag="wk")
            nc.sync.dma_start(wk_sb[:], wkT.rearrange("(o p) c -> p o c", p=P))
            wq_sb = big.tile([P, CIN // P, CG], bf16, tag="wq")
            nc.sync.dma_start(wq_sb[:], wqT.rearrange("(o p) c -> p o c", p=P))
            bq_sb = const.tile([P, CG // P], f32, tag="bq")
            nc.sync.dma_start(bq_sb[:], bq.rearrange("(s p) -> p s", p=P))
            bk_sb = const.tile([P, CG // P], f32, tag="bk")
            nc.sync.dma_start(bk_sb[:], bk.rearrange("(s p) -> p s", p=P))
            xT_sb = big.tile([P, CIN // P, S], bf16, tag="xT")
            xT_r = xT.rearrange("(o p) t -> p o t", p=P)
            for t in range(NQC):
                tsl = slice(t * QC, (t + 1) * QC)
                nc.sync.dma_start(xT_sb[:, :, tsl], xT_r[:, :, tsl])
            wv_sb = big.tile([P, CIN // P, CG], bf16, tag="wv")
            nc.sync.dma_start(wv_sb[:], wvT.rearrange("(o p) c -> p o c", p=P))
            wo_sb = big.tile([P, CG // P, CIN], bf16, tag="wo")
            nc.sync.dma_start(wo_sb[:], woT.rearrange("(o p) c -> p o c", p=P))
            bo_sb = const.tile([P, CIN // P], f32, tag="bo")
            nc.sync.dma_start(bo_sb[:], bo.rearrange("(s p) -> p s", p=P))
            sel_sb = const.tile([2, 2 * P], f16, tag="sel2")
            nc.sync.dma_start(sel_sb[:], sel2[:])

            # ---------- projections ----------
            qT_sb = big.tile([P, CG // P, S], bf16, tag="qT")
            kT_sb = big.tile([P, CG // P, S], bf16, tag="kT")
            # v token-major with a ones column per head (for Z)
            v_sb = big.tile([P, NKC, NH, D + 1], bf16, tag="v")
            nc.vector.memset(v_sb[:, :, :, D:D + 1], 1.0)

            # channel-major qT/kT projection for one (cout-slice, token range)
            def proj_qk(dst, w, b, s, t, half=None):
                pj = sp_tile()
                tsl = slice(t * QC + (0 if half == 1 else 0),
                            t * QC + (QC // 2 if half == 0 else QC))
                if half == 1:
                    tsl = slice(t * QC + QC // 2, (t + 1) * QC)
                n = tsl.stop - tsl.start
                for ci in range(CIN // P):
                    nc.tensor.matmul(
                        pj[:, 0, :n],
                        w[:, ci, s * P:(s + 1) * P],
                        xT_sb[:, ci, tsl],
                        start=(ci == 0),
                        stop=(ci == CIN // P - 1),
                    )
                if EXP_SPLIT > 0:
                    # exps split onto DVE -> keep its FIFO thin: evac on ACT
                    nc.scalar.activation(
                        dst[:, s, tsl], pj[:, 0, :n],
                        mybir.ActivationFunctionType.Identity,
                        bias=b[:, s:s + 1], scale=1.0,
                    )
                else:
                    nc.vector.tensor_scalar_add(
                        dst[:, s, tsl], pj[:, 0, :n], b[:, s:s + 1],
                    )

            # token-major v for one 128-token slice and one head-pair (so
            # pair 1's v work overlaps pair 1's attention); bv is folded
            # into bo host-side (probs sum to 1 after normalization, so
            # Wo @ (ctx/Z + bv) = Wo @ (ctx/Z) + const)
            def proj_v(t, vp):
                pj = sp_tile()
                csl = slice(vp * 2 * D, (vp + 1) * 2 * D)
                for ci in range(CIN // P):
                    nc.tensor.matmul(
                        pj[:, 0, csl],
                        xT_sb[:, ci, t * P:(t + 1) * P],
                        wv_sb[:, ci, csl],
                        start=(ci == 0),
                        stop=(ci == CIN // P - 1),
                    )
                # GPSIMD can't read PSUM; ScalarE is the cheapest psum evac
                # and has idle capacity exactly here (qc0 starves on exp)
                nc.scalar.copy(
                    v_sb[:, t, 2 * vp:2 * (vp + 1), :D],
                    pj[:, 0, csl].rearrange("p (h d) -> p h d", d=D),
                )

            # Minimal prefix for the very first score group (kT on the first
            # half-chunk only), then weave the rest demand-driven into the
            # qc-0 score slots.
            proj_qk(kT_sb, wk_sb, bk_sb, 0, 0, half=0)
            proj_qk(qT_sb, wq_sb, bq_sb, 0, 0)

            def mk(kind, s, t, half=None):
                if kind == "v":
                    return lambda: proj_v(t, s)
                if kind == "k":
                    return lambda: proj_qk(kT_sb, wk_sb, bk_sb, s, t, half)
                return lambda: proj_qk(qT_sb, wq_sb, bq_sb, s, t)

            # keys: ("v", vp, t) / ("k", s, tkey) / ("q", s, t); a kT item
            # with tkey covers key columns up to (tkey+1)*QC
            pend = []   # (key, emit_fn) in drain order
            pend.append((("v", 0, 0), mk("v", 0, 0)))
            pend.append((("v", 0, 1), mk("v", 0, 1)))
            pend.append((("k", 0, 0.5), mk("k", 0, 0, 1)))
            for t in range(1, NQC):
                for vt in (4 * t - 2, 4 * t - 1, 4 * t, 4 * t + 1):
                    if vt < NKC:
                        pend.append((("v", 0, vt), mk("v", 0, vt)))
                pend.append((("k", 0, t), mk("k", 0, t)))
            for vt in (NKC - 2, NKC - 1):
                pend.append((("v", 0, vt), mk("v", 0, vt)))
            # pair-1 items: its kT/qT slices and the v halves for heads 2,3
            pend.append((("k", 1, 0), mk("k", 1, 0)))
            pend.append((("q", 1, 0), mk("q", 1, 0)))
            for t in range(NQC):
                for vt in (4 * t, 4 * t + 1, 4 * t + 2, 4 * t + 3):
                    pend.append((("v", 1, vt), mk("v", 1, vt)))
                if t + 1 < NQC:
                    pend.append((("k", 1, t + 1), mk("k", 1, t + 1)))
            # dedup (keep first occurrence)
            seen = set()
            pend = [p for p in pend if not (p[0] in seen or seen.add(p[0]))]

            def proj_ensure(pair, kcg, phase):
                t_need = (2 * kcg + 1) / 4.0
                v_need = 2 * kcg + 1

                def needed(key):
                    if key[0] == "v":
                        return (phase == "pv" and key[1] == pair
                                and key[2] <= v_need)
                    if phase != "scores":
                        return False
                    if key[0] == "k":
                        return key[1] == pair and key[2] <= t_need
                    return key[1] == pair   # qT slice for this pair
                while pend and any(needed(k) for k, _ in pend):
                    k, f = pend.pop(0)
                    f()

            def proj_drain(n=1, allow_pair1=True):
                for _ in range(n):
                    if not pend:
                        return
                    if not allow_pair1 and pend[0][0][1] == 1:
                        return
                    pend.pop(0)[1]()

            # ---------- attention ----------
            ctx_raw = big.tile([P, CG // P, S], bf16, tag="ctxr")
            ctx_nrm = big.tile([P, CG // P, S], bf16, tag="ctxn")
            outT_sb = big.tile([P, CIN // P, S], bf16, tag="outT")

            # deferred work (closures), emitted one-per-score-slot so PE
            # never head-of-line blocks on tail dependencies
            tail_q = []
            exp_idx = [0]

            def emit_tail_some(n=1):
                for _ in range(n):
                    if tail_q:
                        tail_q.pop(0)()

            def emit_tail_all():
                while tail_q:
                    tail_q.pop(0)()

            def make_pair_tail(qc, pair, zstack, zrows=None):
                """normalize pair's two heads: recip(Z) -> broadcast -> mult.
                With zrows (last pair): skip the z-stack DMAs (~1.5us sem
                latency each) and run per-row recips directly."""
                qsl = slice(qc * QC, (qc + 1) * QC)
                items = []
                nrow = 1 if zrows is not None else 2
                rz_f32 = [rzsp.tile([nrow, QC], mybir.dt.float32,
                                    tag=f"rzf{nrow}_{i}", name=f"rzf{i}")
                          for i in range(2 if zrows is not None else 1)]
                rz_t = [rzsp.tile([nrow, QC], mybir.dt.float16,
                                  tag=f"rz{nrow}_{i}", name=f"rz{i}")
                        for i in range(2 if zrows is not None else 1)]

                def t_recip():
                    with nc.allow_low_precision(
                            reason="1/Z approx (~18 bits): Z ~ O(2048)"):
                        if zrows is not None:
                            for i in range(2):
                                nc.vector.reciprocal_approx_fast(
                                    rz_f32[i][:], zrows[i][:])
                                nc.gpsimd.tensor_copy(rz_t[i][:], rz_f32[i][:])
                        else:
                            nc.vector.reciprocal_approx_fast(
                                rz_f32[0][:], zstack[:])
                            nc.gpsimd.tensor_copy(rz_t[0][:], rz_f32[0][:])
                items.append(t_recip)

                for i in range(2):
                    def t_norm(i=i):
                        h = 2 * pair + i
                        hp, hs = D * (h % 2), h // 2
                        bc = tp_tile()
                        if zrows is not None:
                            nc.tensor.matmul(
                                bc[:, 0, :],
                                sel_sb[0:1, 0:P],
                                rz_t[i][:],
                                start=True, stop=True,
                            )
                        else:
                            nc.tensor.matmul(
                                bc[:, 0, :],
                                sel_sb[:, i * P:(i + 1) * P],
                                rz_t[0][:],
                                start=True, stop=True,
                            )
                        sl = (slice(hp, hp + D), hs, qsl)
                        nc.vector.tensor_tensor(
                            ctx_nrm[sl], ctx_raw[sl], bc[hp:hp + D, 0, :],
                            mybir.AluOpType.mult,
                        )
                    items.append(t_norm)
                return items

            def make_out_tail(qc, last=False):
                qsl = slice(qc * QC, (qc + 1) * QC)
                items = []
                for oc in range(CIN // P):
                    def t_oproj(oc=oc):
                        # last qc: pv banks are free, rotate through them so
                        # the final (unoverlapped) tail pipelines
                        if last:
                            opap = pvp.tile([P, QC], mybir.dt.float32,
                                            tag="pv", name="op")[:, :]
                        else:
                            opap = tp_tile()[:, 0, :]
                        for s in range(CG // P):
                            nc.tensor.matmul(
                                opap,
                                wo_sb[:, s, oc * P:(oc + 1) * P],
                                ctx_nrm[:, s, qsl],
                                start=(s == 0),
                                stop=(s == CG // P - 1),
                            )
                        nc.vector.tensor_scalar_add(
                            outT_sb[:, oc, qsl], opap,
                            bo_sb[:, oc:oc + 1],
                        )
                        nc.sync.dma_start(
                            outT.rearrange("(o p) t -> p o t", p=P)[:, oc, qsl],
                            outT_sb[:, oc, qsl],
                        )
                    items.append(t_oproj)
                return items

            for qc in range(NQC):
                qsl = slice(qc * QC, (qc + 1) * QC)
                for pair in range(2):
                    pvs = [pvp.tile([P, QC], mybir.dt.float32, tag="pv",
                                    name=f"pv{i}") for i in range(2)]
                    # Z rows for this pair, stacked on partitions 0..1 in
                    # SBUF (via tiny DMAs; engines need 32-aligned bases)
                    last_pair = (qc == NQC - 1 and pair == 1)
                    if not last_pair:
                        zstack = zsp.tile([2, QC], mybir.dt.float32, tag="z")
                    else:
                        zstack = None
                    # software pipeline: PV runs one kcg behind scores/exp so
                    # the PE never stalls on the exp round trip
                    prev_pv = None
                    for kcg in range(NKC // 2):
                        if qc == 0:
                            proj_ensure(pair, kcg, "scores")
                        # two kc chunks of scores in 64-row tiling mode:
                        # heads 2p/2p+1 sit at partition bases 0/64 -> array
                        # tiles T0/T8 execute their matmuls concurrently
                        sts = []
                        for j in range(2):
                            kc = 2 * kcg + j
                            st = sp_tile()
                            for i in range(2):
                                h = 2 * pair + i
                                hp, hs = D * (h % 2), h // 2
                                nc.tensor.matmul(
                                    st[:, i, :],
                                    kT_sb[hp:hp + D, hs, kc * P:(kc + 1) * P],
                                    qT_sb[hp:hp + D, hs, qsl],
                                    start=True, stop=True,
                                    tile_position=(hp, 0),
                                )
                            sts.append(st)
                        pts = []
                        for j in range(2):
                            pt = ptp.tile([P, 2, QC], bf16, tag="pt",
                                          name=f"pt{j}")
                            # scores arrive pre-scaled by 1/128 (Wq host
                            # prescale); 16*s = s_true/8.  Roughly 1/3 of
                            # the exp stream runs on the DVE via the custom
                            # poly op so ACT stops being the critical path
                            # (skip DVE in qc0 pair0: DVE is evac-busy and
                            # ACT starved there).
                            nonlocal_idx = exp_idx[0]
                            exp_idx[0] += 1
                            use_dve = (EXP_SPLIT > 0
                                       and nonlocal_idx % EXP_SPLIT == 1
                                       and not (qc == 0 and pair == 0))
                            if use_dve:
                                nc.vector._custom_dve(
                                    exp_op, out=pt[:], in0=sts[j][:],
                                    s0=1.0, s1=1.0, imm2=0.5,
                                )
                            else:
                                nc.scalar.activation(
                                    pt[:], sts[j][:],
                                    mybir.ActivationFunctionType.Exp,
                                    bias=0.0, scale=16.0,
                                )
                            pts.append(pt)
                        def emit_pv(pts, kcg):
                            for j in range(2):
                                kc = 2 * kcg + j
                                for i in range(2):
                                    h = 2 * pair + i
                                    nc.tensor.matmul(
                                        pvs[i][:D + 1, :],
                                        v_sb[:, kc, h, :],
                                        pts[j][:, i, :],
                                        start=(kc == 0),
                                        stop=(kc == NKC - 1),
                                    )
                        if prev_pv is not None:
                            if qc == 0:
                                proj_ensure(pair, prev_pv[1], "pv")
                            emit_pv(*prev_pv)
                        prev_pv = (pts, kcg)
                        if qc == 0:
                            proj_drain(1, allow_pair1=(pair == 1 or kcg >= 6))
                        emit_tail_some(2)
                    if qc == 0:
                        proj_ensure(pair, prev_pv[1], "pv")
                    emit_pv(*prev_pv)
                    zrows = []
                    for i in range(2):
                        h = 2 * pair + i
                        hp, hs = D * (h % 2), h // 2
                        # stash Z (ACT) and unnormalized ctxT (DVE) in
                        # parallel, freeing the pv bank
                        z_row = zsp.tile([1, QC], mybir.dt.float32, tag="zr")
                        nc.scalar.copy(z_row[:], pvs[i][D:D + 1, :])
                        zrows.append(z_row)
                        if not last_pair:
                            nc.sync.dma_start(zstack[i:i + 1, :], z_row[:])
                        nc.vector.tensor_copy(
                            ctx_raw[hp:hp + D, hs, qsl], pvs[i][:D, :],
                        )
                    if pair == 0 and qc + 1 < NQC:
                        # produce qT for the next qc during this qc's pair 1
                        for s in range(CG // P):
                            tail_q.append(mk("q", s, qc + 1))
                    tail_q.extend(make_pair_tail(
                        qc, pair, zstack, zrows=zrows if last_pair else None))
                tail_q.extend(make_out_tail(qc, last=(qc == NQC - 1)))

            emit_tail_all()

    nc.compile()
    return nc


def _get_nc():
    if "nc" not in _CACHE:
        _CACHE["nc"] = _build_nc()
    return _CACHE["nc"]


SCALE_Q = 1.0 / 128.0   # host prescale on Wq/bq; exp uses ACT scale=16


def make_in_maps(query_states, Wq, bq, Wk, bk, Wv, bv, Wo, bo):
    """Host-side shard + layout prep. core c: batch c//2, head-group c%2."""
    bf = ml_dtypes.bfloat16
    x = np.asarray(query_states, np.float32)
    B = x.shape[0]
    in_maps = []
    xT_all = [np.ascontiguousarray(x[b].T).astype(bf) for b in range(B)]
    Wo32 = np.asarray(Wo, np.float32)
    bv32 = np.asarray(bv, np.float32)
    bo32 = np.asarray(bo, np.float32)
    w_sl = {}
    for g in range(2):
        c0, c1 = CG * g, CG * (g + 1)
        bo_eff = (bo32.copy() if g == 0 else np.zeros(CIN, np.float32))
        bo_eff += Wo32[:, c0:c1] @ bv32[c0:c1]
        w_sl[g] = dict(
            wqT=np.ascontiguousarray(
                np.asarray(Wq, np.float32)[c0:c1, :].T * SCALE_Q).astype(bf),
            wkT=np.ascontiguousarray(np.asarray(Wk, np.float32)[c0:c1, :].T).astype(bf),
            wvT=np.ascontiguousarray(np.asarray(Wv, np.float32)[c0:c1, :].T).astype(bf),
            woT=np.ascontiguousarray(Wo32[:, c0:c1].T).astype(bf),
            bq=np.ascontiguousarray(np.asarray(bq, np.float32)[c0:c1] * SCALE_Q),
            bk=np.ascontiguousarray(np.asarray(bk, np.float32)[c0:c1]),
            bo=bo_eff,
        )
    sel2 = np.zeros((2, 2 * P), np.float16)
    for i in range(2):
        sel2[i, i * P:(i + 1) * P] = 1.0
    for c in range(8):
        b, g = c // 2, c % 2
        m = {"xT": xT_all[b], "sel2": sel2}
        m.update(w_sl[g])
        in_maps.append(m)
    return in_maps


def gather_output(results):
    """Sum head-group partials per batch and transpose back to [B, S, C]."""
    B = 4
    out = np.empty((B, S, CIN), np.float32)
    for b in range(B):
        acc = (results[2 * b]["outT"].astype(np.float32)
               + results[2 * b + 1]["outT"].astype(np.float32))
        out[b] = acc.T
    return out


def kernel(query_states, Wq, bq, Wk, bk, Wv, bv, Wo, bo,
           attention_biases=None, bias_idxs=None, **_unused):
    # attention_biases/bias_idxs: bias_idxs is ones(49,49), so the gathered
    # bias is constant per head -> softmax-invariant -> no-op. Unused.
    from concourse.bass_utils import run_bass_kernel_spmd
    nc = _get_nc()
    in_maps = make_in_maps(query_states, Wq, bq, Wk, bk, Wv, bv, Wo, bo)
    res = run_bass_kernel_spmd(nc, in_maps, core_ids=list(range(8)))
    return gather_output(res.results)
